# revision 25
# baseline (speedup 1.0000x reference)
"""Trainium2 Bass kernel for CAttention:
    k      = einsum('bcit,i->bct', x, alpha)
    scores = einsum('bct,ts,bds->bcd', k, Wc, k)
    att    = softmax(scores, axis=-1)
    out    = einsum('bci,bint->bcnt', att, x)

Sharding: data-parallel over batch B=64 across 8 NeuronCores (8 batches/core).

fp16 streaming: x is cast to fp16 on host (DMA-in halves vs f32) and the
output is written fp16 (DMA-out halves), converted back to f32 on host.
The score chain stays accurate enough (validated ~1e-2 max-metric vs the
2e-2 gate) because products/partials accumulate through a 6-level fp16
tree with the last level and everything downstream (kT, Wc, scores,
softmax) in f32.

Per-core layout (per batch b):
    X SBUF tile [128, 8192] fp16: partition p = j*8 + d (j in [0,16) =
    n-chunk, d in [0,8) = channel), free q = n2*64 + t with n = j*128+n2.

    k-path : scr = X * acx (DVE fp16 2x; acx = alpha pre-expanded to
             [128, 8192] on host), 6 fp16 tree levels to 128 wide, last
             level adds to f32 s64[P, 64]; kT[t,d] via f32 PE matmul
             (sums the 16 j-chunks exactly).
    scores : V = Wc @ kT (PE f32); scores = kT.T @ V (PE f32)
    softmax: unnormalized exp on ACT (accum row-sum), 1/sum via DVE
             reciprocal; att = e * (1/sum) folded in an ACT scale-copy to
             fp16 (so the PSUM mix evacuation is a plain copy).
    mix    : block-diag(att^T) [128,128] fp16 stationary; 16 fp16 matmuls
             of 512 into [P,1024] PSUM tiles (2 banks each, 3 bufs)
    out    : ACT copies PSUM -> fp16 staging, gpsimd (Pool) SWDGE rings
             DMA quarters out so the ACT sequencer only does compute.

Emission is software-pipelined one batch deep: phase_a(b+1)'s big DVE
work is enqueued before phase_b(b)'s small-path DVE ops so the DVE never
stalls waiting on the PE/ACT score chain.  Input stream rides the SP
HWDGE ring; constants ride the ACT ring; output uses gpsimd SWDGE.
"""

import sys

for _p in ("/opt/trn_rl_repo",):
    if _p not in sys.path:
        sys.path.insert(0, _p)

import numpy as np

B, C, N, T = 64, 8, 2048, 64
NCORES = 8
BS = B // NCORES          # batches per core
J = 16                    # n-chunks on partitions
N2 = N // J               # 128, n-extent in free dim
P = J * C                 # 128 partitions
F = N2 * T                # 8192 free elems
QW = 512                  # matmul free width (one PSUM bank)
EW = 1024                 # evacuation width (two PSUM banks)
OW = 2048                 # out-staging quarter width

_PROGRAM_CACHE = {}


def _build_program():
    from contextlib import ExitStack

    import concourse.bacc as bacc
    from concourse import mybir, tile

    fp32 = mybir.dt.float32
    fp16 = mybir.dt.float16
    nc = bacc.Bacc("TRN2", target_bir_lowering=False, debug=False)

    xs = nc.dram_tensor("xs", [BS, C, N, T], fp16, kind="ExternalInput").ap()
    acx = nc.dram_tensor("acx", [P, N2], fp16, kind="ExternalInput").ap()
    # a32: sel[:, 0:8] | wcT[0:64, 8:72] | id8[0:8, 72:80] | rep32[0:8, 80:208]
    a32 = nc.dram_tensor("a32", [P, 208], fp32, kind="ExternalInput").ap()
    # a16: rep[0:8, 0:128] | mask[:, 128:256]
    a16 = nc.dram_tensor("a16", [P, 256], fp16, kind="ExternalInput").ap()
    out = nc.dram_tensor("out", [BS, C, N, T], fp16, kind="ExternalOutput").ap()

    Exp = mybir.ActivationFunctionType.Exp
    Copy = mybir.ActivationFunctionType.Copy
    ADD = mybir.AluOpType.add
    MULT = mybir.AluOpType.mult

    with tile.TileContext(nc) as tc, ExitStack() as ctx:
        cpool = ctx.enter_context(tc.tile_pool(name="const", bufs=1))
        xpool = ctx.enter_context(tc.tile_pool(name="x", bufs=5))
        scrpool = ctx.enter_context(tc.tile_pool(name="scr", bufs=2))
        opool = ctx.enter_context(tc.tile_pool(name="o", bufs=12))
        spool = ctx.enter_context(tc.tile_pool(name="small", bufs=4))
        bdpool = ctx.enter_context(tc.tile_pool(name="bd", bufs=4))
        mixp = ctx.enter_context(tc.tile_pool(name="mixp", bufs=3, space="PSUM"))
        psmall = ctx.enter_context(tc.tile_pool(name="psmall", bufs=2, space="PSUM"))

        # constants ride the ACT HWDGE ring so the SP ring starts batch 0's
        # X read immediately; alpha arrives as a small [P, N2] tile and is
        # expanded on-device (saves 2 MB of HBM in-traffic)
        ac_t = cpool.tile([P, N2], fp16)
        nc.scalar.dma_start(ac_t[:], acx)
        a32_t = cpool.tile([P, 208], fp32)
        nc.scalar.dma_start(a32_t[:], a32)
        a16_t = cpool.tile([P, 256], fp16)
        nc.scalar.dma_start(a16_t[:], a16)
        sel_t = a32_t[:, 0:8]
        wcT_t = a32_t[:T, 8:72]
        id8_t = a32_t[:C, 72:80]
        rep32_t = a32_t[:C, 80:208]
        rep_t = a16_t[:C, 0:128]
        mask_t = a16_t[:, 128:256]
        # acx[p, n2*T + t] = alpha-per-partition-chunk, broadcast over t so
        # the per-batch multiply is a fully packed fp16 op (2x DVE mode)
        acx_t = cpool.tile([P, F], fp16)
        for q in range(4):
            n2s = N2 // 4
            nc.scalar.copy(
                acx_t[:, q * n2s * T : (q + 1) * n2s * T].rearrange(
                    "p (n2 t) -> p n2 t", t=T
                ),
                ac_t[:, q * n2s : (q + 1) * n2s]
                .rearrange("p (n2 x) -> p n2 x", x=1)
                .to_broadcast([P, n2s, T]),
            )

        NQ = 4                # DMA quarter granularity
        FQ = F // NQ
        fence_t = cpool.tile([1, NQ], fp16)

        def dma_in(b):
            # quarter-granular so the first compute gates on 0.5 MB, not on
            # the whole prefetch burst
            X = xpool.tile([P, F], fp16, tag="X")
            xb = xs[b].rearrange("d (j n2) t -> j d (n2 t)", j=J)
            for q in range(NQ):
                nc.sync.dma_start(X[:, q * FQ : (q + 1) * FQ], xb[:, :, q * FQ : (q + 1) * FQ])
            return X

        def dma_fence(X):
            # prologue only: parks the SP queue until X has fully landed, so
            # the first batches' transfers don't round-robin with the whole
            # prefetch burst at the DMA engines (halves batch 0's latency)
            nc.sync.dma_start(fence_t[:1, :], X[0:1, 0 : NQ * FQ : FQ])

        def mult(b, X):
            # alpha-weighted product (fp16 2x DVE); quarter-granular for the
            # first batches (DMA-arrival gating), one op once the prefetch
            # runs ahead (saves ~0.5us/batch of DVE dispatch overhead)
            scr = scrpool.tile([P, F], fp16, tag="scr")
            if b < 2:
                for q in range(NQ):
                    s = slice(q * FQ, (q + 1) * FQ)
                    nc.vector.tensor_tensor(
                        out=scr[:, s], in0=X[:, s], in1=acx_t[:, s], op=MULT
                    )
            else:
                nc.vector.tensor_tensor(out=scr[:], in0=X[:], in1=acx_t[:], op=MULT)
            return scr

        def tree(b, scr):
            # contiguous in-place fp16 tree; last level in f32 (kills the
            # largest fp16 rounding term before the exact PE j-sum)
            w = F // 2
            while w >= 2 * T:
                nc.vector.tensor_tensor(
                    out=scr[:, :w], in0=scr[:, :w], in1=scr[:, w : 2 * w], op=ADD
                )
                w //= 2
            s64 = spool.tile([P, T], fp32, tag="s64")
            nc.vector.tensor_tensor(
                out=s64[:], in0=scr[:, :T], in1=scr[:, T : 2 * T], op=ADD
            )
            return s64

        def chain_a(b, s64):
            """k -> V -> scores (PE matmuls with DVE PSUM->SBUF hops)."""
            # kT[t, d] = sum_j s64[(j,d), t]  (f32 matmul, tiny ap)
            kT_ps = psmall.tile([T, C], fp32, tag="ps")
            nc.tensor.matmul(kT_ps[:], lhsT=s64[:], rhs=sel_t, start=True, stop=True)
            kT_sb = spool.tile([T, C], fp32, tag="kTsb")
            nc.vector.tensor_scalar_add(kT_sb[:], kT_ps[:], 0.0)

            # V[t, d] = sum_s Wc[t, s] k[d, s]
            v_ps = psmall.tile([T, C], fp32, tag="ps")
            nc.tensor.matmul(v_ps[:], lhsT=wcT_t, rhs=kT_sb[:], start=True, stop=True)
            v_sb = spool.tile([T, C], fp32, tag="vsb")
            nc.vector.tensor_scalar_add(v_sb[:], v_ps[:], 0.0)

            # scores[c, d] = sum_t k[c, t] V[t, d]
            sc_ps = psmall.tile([C, C], fp32, tag="ps")
            nc.tensor.matmul(sc_ps[:], lhsT=kT_sb[:], rhs=v_sb[:], start=True, stop=True)
            return sc_ps

        def chain_b(b, sc_ps):
            """exp (the only ACT op in the chain) + normalization on DVE.
            Unnormalized exp saturates for scores ~|100| exactly like the
            f32 baseline; normalize in f32 BEFORE the fp16 cast so only
            att in [0,1] enters the fp16 mix path."""
            e_sb = spool.tile([C, C], fp32, tag="esb")
            ssum = spool.tile([C, 1], fp32, tag="ssum")
            nc.scalar.activation(e_sb[:], sc_ps[:], Exp, accum_out=ssum[:])
            rcp = spool.tile([C, 1], fp32, tag="rcp")
            nc.vector.reciprocal(rcp[:], ssum[:])
            att_sb = spool.tile([C, C], fp32, tag="attsb")
            nc.vector.tensor_scalar_mul(att_sb[:], e_sb[:], rcp[:])
            return att_sb

        def chain_c(b, att_sb):
            """Replicate att^T to all j-blocks:
            bd[(j,d), (j',c)] = [j==j'] att[c,d]."""
            eT_ps = psmall.tile([C, C], fp32, tag="ps")
            nc.tensor.transpose(eT_ps[:], att_sb[:], id8_t)
            eT_sb = spool.tile([C, C], fp16, tag="eTsb")
            nc.vector.tensor_scalar_add(eT_sb[:], eT_ps[:], 0.0)
            er_ps = psmall.tile([P, C], fp32, tag="ps")
            nc.tensor.matmul(er_ps[:], lhsT=rep_t, rhs=eT_sb[:], start=True, stop=True)
            bd = bdpool.tile([P, P], fp16, tag="bd")
            nc.vector.tensor_tensor(
                out=bd[:].rearrange("p (j c) -> p j c", j=J),
                in0=mask_t.rearrange("p (j c) -> p j c", j=J),
                in1=er_ps[:].rearrange("p (x c) -> p x c", x=1).to_broadcast([P, J, C]),
                op=MULT,
            )
            return bd

        def mix_quarter(b, X, bd, qs):
            """One output quarter: fp16 matmuls into [P,1024] PSUM tiles,
            ACT plain-copies to fp16 staging (normalization already folded
            into bd), a Pool-issued SWDGE DMA drains the quarter.  Two of
            the eight evacuation chunks per batch ride the otherwise-idle
            GpSimd so the ACT queue (the mix rate-setter) sheds ~2.2us."""
            out_b = out[b].rearrange("c (j n2) t -> j c (n2 t)", j=J)
            ost = opool.tile([P, OW], fp16, tag="ost")
            for h in range(OW // EW):
                mp = mixp.tile([P, EW], fp32, tag="mix")
                for g in range(EW // QW):
                    q0 = qs * OW + h * EW + g * QW
                    nc.tensor.matmul(
                        mp[:, g * QW : (g + 1) * QW],
                        lhsT=bd[:],
                        rhs=X[:, q0 : q0 + QW],
                        start=True,
                        stop=True,
                    )
                nc.scalar.copy(ost[:, h * EW : (h + 1) * EW], mp[:])
            nc.gpsimd.dma_start(
                out_b[:, :, qs * OW : (qs + 1) * OW],
                ost[:],
            )

        # Two-deep software pipeline, scheduled so each engine's in-order
        # queue almost never parks: iteration i mixes batch i (PE+ACT),
        # reduces batch i+1 (DVE tree then the score chain, whose PE/ACT
        # hops are interleaved between mix quarters at points where their
        # inputs are already computed), and streams batch i+2 in (SP DMA,
        # with the big alpha-multiply emitted last, after the DMA landed).
        Xs = {0: dma_in(0)}
        dma_fence(Xs[0])
        Xs[1] = dma_in(1)
        dma_fence(Xs[1])
        scrs = {0: mult(0, Xs[0])}
        s64s = {0: tree(0, scrs.pop(0))}
        bds = {0: chain_c(0, chain_b(0, chain_a(0, s64s.pop(0))))}
        scrs[1] = mult(1, Xs[1])
        for i in range(BS):
            if i + 2 < BS:
                Xs[i + 2] = dma_in(i + 2)
            nxt = i + 1 < BS
            if nxt:
                s64s[i + 1] = tree(i + 1, scrs.pop(i + 1))
            mix_quarter(i, Xs[i], bds[i], 0)
            mix_quarter(i, Xs[i], bds[i], 1)
            if nxt:
                sc_ps = chain_a(i + 1, s64s.pop(i + 1))
                att_sb = chain_b(i + 1, sc_ps)
            mix_quarter(i, Xs[i], bds[i], 2)
            if nxt:
                bds[i + 1] = chain_c(i + 1, att_sb)
            mix_quarter(i, Xs[i], bds.pop(i), 3)
            Xs.pop(i)
            if i + 2 < BS:
                scrs[i + 2] = mult(i + 2, Xs[i + 2])

    nc.compile()
    return nc


def _host_constants(Wc: np.ndarray, alpha: np.ndarray):
    # acx[(j*8+d), n2*64+t] = alpha[j*128+n2]  (pre-expanded so the DVE
    # multiply is a packed elementwise op eligible for the 2x fp16 mode)
    a = np.asarray(alpha, dtype=np.float32).reshape(J, N2)
    acx = np.repeat(a, C, axis=0).astype(np.float16)      # [(j,d), n2]
    # sel[(j*8+d), d'] = 1 if d == d'
    sel = np.tile(np.eye(C, dtype=np.float32), (J, 1))
    # rep[c', j*8+c] = 1 if c == c'  (partition replication)
    rep32 = np.tile(np.eye(C, dtype=np.float32), (1, J))
    a32 = np.zeros((P, 208), dtype=np.float32)
    a32[:, 0:8] = sel
    a32[:T, 8:72] = np.asarray(Wc, dtype=np.float32).T
    a32[:C, 72:80] = np.eye(C, dtype=np.float32)
    a32[:C, 80:208] = rep32
    # mask[(j,d), (j',c)] = 1 if j == j'
    mask = np.kron(np.eye(J, dtype=np.float16), np.ones((C, C), dtype=np.float16))
    a16 = np.zeros((P, 256), dtype=np.float16)
    a16[:C, 0:128] = rep32.astype(np.float16)
    a16[:, 128:256] = mask
    return {
        "acx": np.ascontiguousarray(acx),
        "a32": a32,
        "a16": a16,
    }


def get_program():
    if "nc" not in _PROGRAM_CACHE:
        _PROGRAM_CACHE["nc"] = _build_program()
    return _PROGRAM_CACHE["nc"]


def run(x, Wc, alpha, trace=False, trace_kwargs=None):
    """Run on 8 cores; returns (full_output, BassKernelResults)."""
    from concourse.bass_utils import run_bass_kernel_spmd

    nc = get_program()
    consts = _host_constants(np.asarray(Wc), np.asarray(alpha))
    xh = np.asarray(x).astype(np.float16)
    in_maps = []
    for r in range(NCORES):
        m = {"xs": np.ascontiguousarray(xh[r * BS : (r + 1) * BS])}
        m.update(consts)
        in_maps.append(m)
    kw = {}
    if trace:
        kw["trace"] = True
        if trace_kwargs:
            kw.update(trace_kwargs)
    res = run_bass_kernel_spmd(nc, in_maps, list(range(NCORES)), **kw)
    out = np.concatenate(
        [np.asarray(res.results[r]["out"]) for r in range(NCORES)], axis=0
    ).astype(np.float32)
    return out, res


def kernel(x, Wc, alpha):
    out, _ = run(x, Wc, alpha)
    return out.astype(np.float32)


# revision 26
# speedup vs baseline: 1.0181x; 1.0181x over previous
"""Trainium2 Bass kernel for CAttention:
    k      = einsum('bcit,i->bct', x, alpha)
    scores = einsum('bct,ts,bds->bcd', k, Wc, k)
    att    = softmax(scores, axis=-1)
    out    = einsum('bci,bint->bcnt', att, x)

Sharding: data-parallel over batch B=64 across 8 NeuronCores (8 batches/core).

fp16 streaming: x is cast to fp16 on host (DMA-in halves vs f32) and the
output is written fp16 (DMA-out halves), converted back to f32 on host.
The score chain stays accurate enough (validated ~1e-2 max-metric vs the
2e-2 gate) because products/partials accumulate through a 6-level fp16
tree with the last level and everything downstream (kT, Wc, scores,
softmax) in f32.

Per-core layout (per batch b):
    X SBUF tile [128, 8192] fp16: partition p = j*8 + d (j in [0,16) =
    n-chunk, d in [0,8) = channel), free q = n2*64 + t with n = j*128+n2.

    k-path : scr = X * acx (DVE fp16 2x; acx = alpha pre-expanded to
             [128, 8192] on host), 6 fp16 tree levels to 128 wide, last
             level adds to f32 s64[P, 64]; kT[t,d] via f32 PE matmul
             (sums the 16 j-chunks exactly).
    scores : V = Wc @ kT (PE f32); scores = kT.T @ V (PE f32)
    softmax: unnormalized exp on ACT (accum row-sum), 1/sum via DVE
             reciprocal; att = e * (1/sum) folded in an ACT scale-copy to
             fp16 (so the PSUM mix evacuation is a plain copy).
    mix    : block-diag(att^T) [128,128] fp16 stationary; 16 fp16 matmuls
             of 512 into [P,1024] PSUM tiles (2 banks each, 3 bufs)
    out    : ACT copies PSUM -> fp16 staging, gpsimd (Pool) SWDGE rings
             DMA quarters out so the ACT sequencer only does compute.

Emission is software-pipelined one batch deep: phase_a(b+1)'s big DVE
work is enqueued before phase_b(b)'s small-path DVE ops so the DVE never
stalls waiting on the PE/ACT score chain.  Input stream rides the SP
HWDGE ring; constants ride the ACT ring; output uses gpsimd SWDGE.
"""

import sys

for _p in ("/opt/trn_rl_repo",):
    if _p not in sys.path:
        sys.path.insert(0, _p)

import numpy as np

B, C, N, T = 64, 8, 2048, 64
NCORES = 8
BS = B // NCORES          # batches per core
J = 16                    # n-chunks on partitions
N2 = N // J               # 128, n-extent in free dim
P = J * C                 # 128 partitions
F = N2 * T                # 8192 free elems
QW = 512                  # matmul free width (one PSUM bank)
EW = 1024                 # evacuation width (two PSUM banks)
OW = 2048                 # out-staging quarter width

_PROGRAM_CACHE = {}


def _build_program():
    from contextlib import ExitStack

    import concourse.bacc as bacc
    from concourse import mybir, tile

    fp32 = mybir.dt.float32
    fp16 = mybir.dt.float16
    nc = bacc.Bacc("TRN2", target_bir_lowering=False, debug=False)

    xs = nc.dram_tensor("xs", [BS, C, N, T], fp16, kind="ExternalInput").ap()
    acx = nc.dram_tensor("acx", [P, N2], fp16, kind="ExternalInput").ap()
    # a32: sel[:, 0:8] | wcT[0:64, 8:72] | id8[0:8, 72:80] | rep32[0:8, 80:208]
    a32 = nc.dram_tensor("a32", [P, 208], fp32, kind="ExternalInput").ap()
    # a16: rep[0:8, 0:128] | mask[:, 128:256]
    a16 = nc.dram_tensor("a16", [P, 256], fp16, kind="ExternalInput").ap()
    out = nc.dram_tensor("out", [BS, C, N, T], fp16, kind="ExternalOutput").ap()

    Exp = mybir.ActivationFunctionType.Exp
    Copy = mybir.ActivationFunctionType.Copy
    ADD = mybir.AluOpType.add
    MULT = mybir.AluOpType.mult

    with tile.TileContext(nc) as tc, ExitStack() as ctx:
        cpool = ctx.enter_context(tc.tile_pool(name="const", bufs=1))
        xpool = ctx.enter_context(tc.tile_pool(name="x", bufs=5))
        scrpool = ctx.enter_context(tc.tile_pool(name="scr", bufs=2))
        opool = ctx.enter_context(tc.tile_pool(name="o", bufs=12))
        spool = ctx.enter_context(tc.tile_pool(name="small", bufs=4))
        bdpool = ctx.enter_context(tc.tile_pool(name="bd", bufs=4))
        mixp = ctx.enter_context(tc.tile_pool(name="mixp", bufs=3, space="PSUM"))
        psmall = ctx.enter_context(tc.tile_pool(name="psmall", bufs=2, space="PSUM"))

        # constants ride the ACT HWDGE ring so the SP ring starts batch 0's
        # X read immediately; alpha arrives as a small [P, N2] tile and is
        # expanded on-device (saves 2 MB of HBM in-traffic)
        ac_t = cpool.tile([P, N2], fp16)
        nc.scalar.dma_start(ac_t[:], acx)
        a32_t = cpool.tile([P, 208], fp32)
        nc.scalar.dma_start(a32_t[:], a32)
        a16_t = cpool.tile([P, 256], fp16)
        nc.scalar.dma_start(a16_t[:], a16)
        sel_t = a32_t[:, 0:8]
        wcT_t = a32_t[:T, 8:72]
        id8_t = a32_t[:C, 72:80]
        rep32_t = a32_t[:C, 80:208]
        rep_t = a16_t[:C, 0:128]
        mask_t = a16_t[:, 128:256]
        # acx[p, n2*T + t] = alpha-per-partition-chunk, broadcast over t so
        # the per-batch multiply is a fully packed fp16 op (2x DVE mode)
        acx_t = cpool.tile([P, F], fp16)
        for q in range(4):
            n2s = N2 // 4
            nc.scalar.copy(
                acx_t[:, q * n2s * T : (q + 1) * n2s * T].rearrange(
                    "p (n2 t) -> p n2 t", t=T
                ),
                ac_t[:, q * n2s : (q + 1) * n2s]
                .rearrange("p (n2 x) -> p n2 x", x=1)
                .to_broadcast([P, n2s, T]),
            )

        NQ = 4                # DMA quarter granularity
        FQ = F // NQ
        fence_t = cpool.tile([1, NQ], fp16)

        def dma_in(b):
            # quarter-granular so the first compute gates on 0.5 MB, not on
            # the whole prefetch burst
            X = xpool.tile([P, F], fp16, tag="X")
            xb = xs[b].rearrange("d (j n2) t -> j d (n2 t)", j=J)
            for q in range(NQ):
                nc.sync.dma_start(X[:, q * FQ : (q + 1) * FQ], xb[:, :, q * FQ : (q + 1) * FQ])
            return X

        def dma_fence(X):
            # prologue only: parks the SP queue until X has fully landed, so
            # the first batches' transfers don't round-robin with the whole
            # prefetch burst at the DMA engines (halves batch 0's latency)
            nc.sync.dma_start(fence_t[:1, :], X[0:1, 0 : NQ * FQ : FQ])

        def mult(b, X):
            # alpha-weighted product (fp16 2x DVE); quarter-granular for the
            # first batches (DMA-arrival gating), one op once the prefetch
            # runs ahead (saves ~0.5us/batch of DVE dispatch overhead)
            scr = scrpool.tile([P, F], fp16, tag="scr")
            if b < 2:
                for q in range(NQ):
                    s = slice(q * FQ, (q + 1) * FQ)
                    nc.vector.tensor_tensor(
                        out=scr[:, s], in0=X[:, s], in1=acx_t[:, s], op=MULT
                    )
            else:
                nc.vector.tensor_tensor(out=scr[:], in0=X[:], in1=acx_t[:], op=MULT)
            return scr

        def tree(b, scr):
            # contiguous in-place fp16 tree; last level in f32 (kills the
            # largest fp16 rounding term before the exact PE j-sum)
            w = F // 2
            while w >= 2 * T:
                nc.vector.tensor_tensor(
                    out=scr[:, :w], in0=scr[:, :w], in1=scr[:, w : 2 * w], op=ADD
                )
                w //= 2
            s64 = spool.tile([P, T], fp32, tag="s64")
            nc.vector.tensor_tensor(
                out=s64[:], in0=scr[:, :T], in1=scr[:, T : 2 * T], op=ADD
            )
            return s64

        def chain_a(b, s64):
            """k -> V -> scores (PE matmuls with DVE PSUM->SBUF hops)."""
            # kT[t, d] = sum_j s64[(j,d), t]  (f32 matmul, tiny ap)
            kT_ps = psmall.tile([T, C], fp32, tag="ps")
            nc.tensor.matmul(kT_ps[:], lhsT=s64[:], rhs=sel_t, start=True, stop=True)
            kT_sb = spool.tile([T, C], fp32, tag="kTsb")
            nc.vector.tensor_scalar_add(kT_sb[:], kT_ps[:], 0.0)

            # V[t, d] = sum_s Wc[t, s] k[d, s]
            v_ps = psmall.tile([T, C], fp32, tag="ps")
            nc.tensor.matmul(v_ps[:], lhsT=wcT_t, rhs=kT_sb[:], start=True, stop=True)
            v_sb = spool.tile([T, C], fp32, tag="vsb")
            nc.vector.tensor_scalar_add(v_sb[:], v_ps[:], 0.0)

            # scores[c, d] = sum_t k[c, t] V[t, d]
            sc_ps = psmall.tile([C, C], fp32, tag="ps")
            nc.tensor.matmul(sc_ps[:], lhsT=kT_sb[:], rhs=v_sb[:], start=True, stop=True)
            return sc_ps

        def chain_b(b, sc_ps):
            """exp (the only ACT op in the chain) + normalization on DVE.
            Unnormalized exp saturates for scores ~|100| exactly like the
            f32 baseline; normalize in f32 BEFORE the fp16 cast so only
            att in [0,1] enters the fp16 mix path."""
            e_sb = spool.tile([C, C], fp32, tag="esb")
            ssum = spool.tile([C, 1], fp32, tag="ssum")
            nc.scalar.activation(e_sb[:], sc_ps[:], Exp, accum_out=ssum[:])
            rcp = spool.tile([C, 1], fp32, tag="rcp")
            nc.vector.reciprocal(rcp[:], ssum[:])
            att_sb = spool.tile([C, C], fp32, tag="attsb")
            nc.vector.tensor_scalar_mul(att_sb[:], e_sb[:], rcp[:])
            return att_sb

        def chain_c(b, att_sb):
            """Replicate att^T to all j-blocks:
            bd[(j,d), (j',c)] = [j==j'] att[c,d]."""
            eT_ps = psmall.tile([C, C], fp32, tag="ps")
            nc.tensor.transpose(eT_ps[:], att_sb[:], id8_t)
            eT_sb = spool.tile([C, C], fp16, tag="eTsb")
            nc.vector.tensor_scalar_add(eT_sb[:], eT_ps[:], 0.0)
            er_ps = psmall.tile([P, C], fp32, tag="ps")
            nc.tensor.matmul(er_ps[:], lhsT=rep_t, rhs=eT_sb[:], start=True, stop=True)
            bd = bdpool.tile([P, P], fp16, tag="bd")
            nc.vector.tensor_tensor(
                out=bd[:].rearrange("p (j c) -> p j c", j=J),
                in0=mask_t.rearrange("p (j c) -> p j c", j=J),
                in1=er_ps[:].rearrange("p (x c) -> p x c", x=1).to_broadcast([P, J, C]),
                op=MULT,
            )
            return bd

        def mix_quarter(b, X, bd, qs):
            """One output quarter: fp16 matmuls into [P,1024] PSUM tiles,
            ACT plain-copies to fp16 staging (normalization already folded
            into bd), a Pool-issued SWDGE DMA drains the quarter.  Two of
            the eight evacuation chunks per batch ride the otherwise-idle
            GpSimd so the ACT queue (the mix rate-setter) sheds ~2.2us."""
            out_b = out[b].rearrange("c (j n2) t -> j c (n2 t)", j=J)
            ost = opool.tile([P, OW], fp16, tag="ost")
            for h in range(OW // EW):
                mp = mixp.tile([P, EW], fp32, tag="mix")
                for g in range(EW // QW):
                    q0 = qs * OW + h * EW + g * QW
                    nc.tensor.matmul(
                        mp[:, g * QW : (g + 1) * QW],
                        lhsT=bd[:],
                        rhs=X[:, q0 : q0 + QW],
                        start=True,
                        stop=True,
                    )
                nc.scalar.copy(ost[:, h * EW : (h + 1) * EW], mp[:])
            nc.gpsimd.dma_start(
                out_b[:, :, qs * OW : (qs + 1) * OW],
                ost[:],
            )

        # Two-deep software pipeline, scheduled so each engine's in-order
        # queue almost never parks: iteration i mixes batch i (PE+ACT),
        # reduces batch i+1 (DVE tree then the score chain, whose PE/ACT
        # hops are interleaved between mix quarters at points where their
        # inputs are already computed), and streams batch i+2 in (SP DMA,
        # with the big alpha-multiply emitted last, after the DMA landed).
        Xs = {0: dma_in(0)}
        dma_fence(Xs[0])
        Xs[1] = dma_in(1)
        scrs = {0: mult(0, Xs[0])}
        s64s = {0: tree(0, scrs.pop(0))}
        bds = {0: chain_c(0, chain_b(0, chain_a(0, s64s.pop(0))))}
        scrs[1] = mult(1, Xs[1])
        for i in range(BS):
            if i + 2 < BS:
                Xs[i + 2] = dma_in(i + 2)
            nxt = i + 1 < BS
            if nxt:
                s64s[i + 1] = tree(i + 1, scrs.pop(i + 1))
            mix_quarter(i, Xs[i], bds[i], 0)
            mix_quarter(i, Xs[i], bds[i], 1)
            if nxt:
                sc_ps = chain_a(i + 1, s64s.pop(i + 1))
                att_sb = chain_b(i + 1, sc_ps)
            mix_quarter(i, Xs[i], bds[i], 2)
            if nxt:
                bds[i + 1] = chain_c(i + 1, att_sb)
            mix_quarter(i, Xs[i], bds.pop(i), 3)
            Xs.pop(i)
            if i + 2 < BS:
                scrs[i + 2] = mult(i + 2, Xs[i + 2])

    nc.compile()
    return nc


def _host_constants(Wc: np.ndarray, alpha: np.ndarray):
    # acx[(j*8+d), n2*64+t] = alpha[j*128+n2]  (pre-expanded so the DVE
    # multiply is a packed elementwise op eligible for the 2x fp16 mode)
    a = np.asarray(alpha, dtype=np.float32).reshape(J, N2)
    acx = np.repeat(a, C, axis=0).astype(np.float16)      # [(j,d), n2]
    # sel[(j*8+d), d'] = 1 if d == d'
    sel = np.tile(np.eye(C, dtype=np.float32), (J, 1))
    # rep[c', j*8+c] = 1 if c == c'  (partition replication)
    rep32 = np.tile(np.eye(C, dtype=np.float32), (1, J))
    a32 = np.zeros((P, 208), dtype=np.float32)
    a32[:, 0:8] = sel
    a32[:T, 8:72] = np.asarray(Wc, dtype=np.float32).T
    a32[:C, 72:80] = np.eye(C, dtype=np.float32)
    a32[:C, 80:208] = rep32
    # mask[(j,d), (j',c)] = 1 if j == j'
    mask = np.kron(np.eye(J, dtype=np.float16), np.ones((C, C), dtype=np.float16))
    a16 = np.zeros((P, 256), dtype=np.float16)
    a16[:C, 0:128] = rep32.astype(np.float16)
    a16[:, 128:256] = mask
    return {
        "acx": np.ascontiguousarray(acx),
        "a32": a32,
        "a16": a16,
    }


def get_program():
    if "nc" not in _PROGRAM_CACHE:
        _PROGRAM_CACHE["nc"] = _build_program()
    return _PROGRAM_CACHE["nc"]


def run(x, Wc, alpha, trace=False, trace_kwargs=None):
    """Run on 8 cores; returns (full_output, BassKernelResults)."""
    from concourse.bass_utils import run_bass_kernel_spmd

    nc = get_program()
    consts = _host_constants(np.asarray(Wc), np.asarray(alpha))
    xh = np.asarray(x).astype(np.float16)
    in_maps = []
    for r in range(NCORES):
        m = {"xs": np.ascontiguousarray(xh[r * BS : (r + 1) * BS])}
        m.update(consts)
        in_maps.append(m)
    kw = {}
    if trace:
        kw["trace"] = True
        if trace_kwargs:
            kw.update(trace_kwargs)
    res = run_bass_kernel_spmd(nc, in_maps, list(range(NCORES)), **kw)
    out = np.concatenate(
        [np.asarray(res.results[r]["out"]) for r in range(NCORES)], axis=0
    ).astype(np.float32)
    return out, res


def kernel(x, Wc, alpha):
    out, _ = run(x, Wc, alpha)
    return out.astype(np.float32)


# revision 27
# speedup vs baseline: 1.1020x; 1.0825x over previous
"""Trainium2 Bass kernel for CAttention:
    k      = einsum('bcit,i->bct', x, alpha)
    scores = einsum('bct,ts,bds->bcd', k, Wc, k)
    att    = softmax(scores, axis=-1)
    out    = einsum('bci,bint->bcnt', att, x)

Sharding: data-parallel over batch B=64 across 8 NeuronCores (8 batches/core).

fp16 streaming: x is cast to fp16 on host (DMA-in halves vs f32) and the
output is written fp16 (DMA-out halves), converted back to f32 on host.
The score chain stays accurate enough (validated ~1e-2 max-metric vs the
2e-2 gate) because products/partials accumulate through a 6-level fp16
tree with the last level and everything downstream (kT, Wc, scores,
softmax) in f32.

Per-core layout (per batch b):
    X SBUF tile [128, 8192] fp16: partition p = j*8 + d (j in [0,16) =
    n-chunk, d in [0,8) = channel), free q = n2*64 + t with n = j*128+n2.

    k-path : scr = X * acx (DVE fp16 2x; acx = alpha pre-expanded to
             [128, 8192] on host), 6 fp16 tree levels to 128 wide, last
             level adds to f32 s64[P, 64]; kT[t,d] via f32 PE matmul
             (sums the 16 j-chunks exactly).
    scores : V = Wc @ kT (PE f32); scores = kT.T @ V (PE f32)
    softmax: unnormalized exp on ACT (accum row-sum), 1/sum via DVE
             reciprocal; att = e * (1/sum) folded in an ACT scale-copy to
             fp16 (so the PSUM mix evacuation is a plain copy).
    mix    : block-diag(att^T) [128,128] fp16 stationary; 16 fp16 matmuls
             of 512 into [P,1024] PSUM tiles (2 banks each, 3 bufs)
    out    : ACT copies PSUM -> fp16 staging, gpsimd (Pool) SWDGE rings
             DMA quarters out so the ACT sequencer only does compute.

Emission is software-pipelined one batch deep: phase_a(b+1)'s big DVE
work is enqueued before phase_b(b)'s small-path DVE ops so the DVE never
stalls waiting on the PE/ACT score chain.  Input stream rides the SP
HWDGE ring; constants ride the ACT ring; output uses gpsimd SWDGE.
"""

import sys

for _p in ("/opt/trn_rl_repo",):
    if _p not in sys.path:
        sys.path.insert(0, _p)

import numpy as np

B, C, N, T = 64, 8, 2048, 64
NCORES = 8
BS = B // NCORES          # batches per core
J = 16                    # n-chunks on partitions
N2 = N // J               # 128, n-extent in free dim
P = J * C                 # 128 partitions
F = N2 * T                # 8192 free elems
QW = 512                  # matmul free width (one PSUM bank)
EW = 1024                 # evacuation width (two PSUM banks)
OW = 2048                 # out-staging quarter width

_PROGRAM_CACHE = {}


def _build_program():
    from contextlib import ExitStack

    import concourse.bacc as bacc
    from concourse import mybir, tile

    fp32 = mybir.dt.float32
    fp16 = mybir.dt.float16
    nc = bacc.Bacc("TRN2", target_bir_lowering=False, debug=False)

    xs = nc.dram_tensor("xs", [BS, C, N, T], fp16, kind="ExternalInput").ap()
    acx = nc.dram_tensor("acx", [P, N2], fp16, kind="ExternalInput").ap()
    # a32: sel[:, 0:8] | wcT[0:64, 8:72] | id8[0:8, 72:80] | rep32[0:8, 80:208]
    a32 = nc.dram_tensor("a32", [P, 208], fp32, kind="ExternalInput").ap()
    # a16: rep[0:8, 0:128] | mask[:, 128:256]
    a16 = nc.dram_tensor("a16", [P, 256], fp16, kind="ExternalInput").ap()
    out = nc.dram_tensor("out", [BS, C, N, T], fp16, kind="ExternalOutput").ap()

    Exp = mybir.ActivationFunctionType.Exp
    Copy = mybir.ActivationFunctionType.Copy
    ADD = mybir.AluOpType.add
    MULT = mybir.AluOpType.mult

    with tile.TileContext(nc) as tc, ExitStack() as ctx:
        cpool = ctx.enter_context(tc.tile_pool(name="const", bufs=1))
        xpool = ctx.enter_context(tc.tile_pool(name="x", bufs=5))
        scrpool = ctx.enter_context(tc.tile_pool(name="scr", bufs=2))
        opool = ctx.enter_context(tc.tile_pool(name="o", bufs=12))
        spool = ctx.enter_context(tc.tile_pool(name="small", bufs=4))
        bdpool = ctx.enter_context(tc.tile_pool(name="bd", bufs=4))
        mixp = ctx.enter_context(tc.tile_pool(name="mixp", bufs=3, space="PSUM"))
        psmall = ctx.enter_context(tc.tile_pool(name="psmall", bufs=2, space="PSUM"))

        # constants ride the ACT HWDGE ring so the SP ring starts batch 0's
        # X read immediately; alpha arrives as a small [P, N2] tile and is
        # expanded on-device (saves 2 MB of HBM in-traffic)
        ac_t = cpool.tile([P, N2], fp16)
        nc.scalar.dma_start(ac_t[:], acx)
        a32_t = cpool.tile([P, 208], fp32)
        nc.scalar.dma_start(a32_t[:], a32)
        a16_t = cpool.tile([P, 256], fp16)
        nc.scalar.dma_start(a16_t[:], a16)
        sel_t = a32_t[:, 0:8]
        wcT_t = a32_t[:T, 8:72]
        id8_t = a32_t[:C, 72:80]
        rep32_t = a32_t[:C, 80:208]
        rep_t = a16_t[:C, 0:128]
        mask_t = a16_t[:, 128:256]
        # acx[p, n2*T + t] = alpha-per-partition-chunk, broadcast over t so
        # the per-batch multiply is a fully packed fp16 op (2x DVE mode)
        acx_t = cpool.tile([P, F], fp16)
        for q in range(4):
            n2s = N2 // 4
            nc.scalar.copy(
                acx_t[:, q * n2s * T : (q + 1) * n2s * T].rearrange(
                    "p (n2 t) -> p n2 t", t=T
                ),
                ac_t[:, q * n2s : (q + 1) * n2s]
                .rearrange("p (n2 x) -> p n2 x", x=1)
                .to_broadcast([P, n2s, T]),
            )

        NQ = 4                # DMA quarter granularity
        FQ = F // NQ
        fence_t = cpool.tile([1, NQ], fp16)

        def dma_in(b):
            # quarter-granular so the first compute gates on 0.5 MB, not on
            # the whole prefetch burst
            X = xpool.tile([P, F], fp16, tag="X")
            xb = xs[b].rearrange("d (j n2) t -> j d (n2 t)", j=J)
            for q in range(NQ):
                nc.sync.dma_start(X[:, q * FQ : (q + 1) * FQ], xb[:, :, q * FQ : (q + 1) * FQ])
            return X

        def dma_fence(X):
            # prologue only: parks the SP queue until X has fully landed, so
            # the first batches' transfers don't round-robin with the whole
            # prefetch burst at the DMA engines (halves batch 0's latency)
            nc.sync.dma_start(fence_t[:1, :], X[0:1, 0 : NQ * FQ : FQ])

        def mult(b, X):
            # alpha-weighted product (fp16 2x DVE), one op per DMA quarter
            scr = scrpool.tile([P, F], fp16, tag="scr")
            for q in range(NQ):
                s = slice(q * FQ, (q + 1) * FQ)
                nc.vector.tensor_tensor(
                    out=scr[:, s], in0=X[:, s], in1=acx_t[:, s], op=MULT
                )
            return scr

        def tree(b, scr):
            # contiguous in-place fp16 tree; last level in f32 (kills the
            # largest fp16 rounding term before the exact PE j-sum)
            w = F // 2
            while w >= 2 * T:
                nc.vector.tensor_tensor(
                    out=scr[:, :w], in0=scr[:, :w], in1=scr[:, w : 2 * w], op=ADD
                )
                w //= 2
            s64 = spool.tile([P, T], fp32, tag="s64")
            nc.vector.tensor_tensor(
                out=s64[:], in0=scr[:, :T], in1=scr[:, T : 2 * T], op=ADD
            )
            return s64

        def chain_a(b, s64):
            """k -> V -> scores (PE matmuls with DVE PSUM->SBUF hops)."""
            # kT[t, d] = sum_j s64[(j,d), t]  (f32 matmul, tiny ap)
            kT_ps = psmall.tile([T, C], fp32, tag="ps")
            nc.tensor.matmul(kT_ps[:], lhsT=s64[:], rhs=sel_t, start=True, stop=True)
            kT_sb = spool.tile([T, C], fp32, tag="kTsb")
            nc.vector.tensor_scalar_add(kT_sb[:], kT_ps[:], 0.0)

            # V[t, d] = sum_s Wc[t, s] k[d, s]
            v_ps = psmall.tile([T, C], fp32, tag="ps")
            nc.tensor.matmul(v_ps[:], lhsT=wcT_t, rhs=kT_sb[:], start=True, stop=True)
            v_sb = spool.tile([T, C], fp32, tag="vsb")
            nc.vector.tensor_scalar_add(v_sb[:], v_ps[:], 0.0)

            # scores[c, d] = sum_t k[c, t] V[t, d]
            sc_ps = psmall.tile([C, C], fp32, tag="ps")
            nc.tensor.matmul(sc_ps[:], lhsT=kT_sb[:], rhs=v_sb[:], start=True, stop=True)
            return sc_ps

        def chain_b(b, sc_ps):
            """exp (the only ACT op in the chain) + normalization on DVE.
            Unnormalized exp saturates for scores ~|100| exactly like the
            f32 baseline; normalize in f32 BEFORE the fp16 cast so only
            att in [0,1] enters the fp16 mix path."""
            e_sb = spool.tile([C, C], fp32, tag="esb")
            ssum = spool.tile([C, 1], fp32, tag="ssum")
            nc.scalar.activation(e_sb[:], sc_ps[:], Exp, accum_out=ssum[:])
            rcp = spool.tile([C, 1], fp32, tag="rcp")
            nc.vector.reciprocal(rcp[:], ssum[:])
            att_sb = spool.tile([C, C], fp32, tag="attsb")
            nc.vector.tensor_scalar_mul(att_sb[:], e_sb[:], rcp[:])
            return att_sb

        def chain_c(b, att_sb):
            """Replicate att^T to all j-blocks:
            bd[(j,d), (j',c)] = [j==j'] att[c,d]."""
            eT_ps = psmall.tile([C, C], fp32, tag="ps")
            nc.tensor.transpose(eT_ps[:], att_sb[:], id8_t)
            eT_sb = spool.tile([C, C], fp16, tag="eTsb")
            nc.vector.tensor_scalar_add(eT_sb[:], eT_ps[:], 0.0)
            er_ps = psmall.tile([P, C], fp32, tag="ps")
            nc.tensor.matmul(er_ps[:], lhsT=rep_t, rhs=eT_sb[:], start=True, stop=True)
            bd = bdpool.tile([P, P], fp16, tag="bd")
            nc.vector.tensor_tensor(
                out=bd[:].rearrange("p (j c) -> p j c", j=J),
                in0=mask_t.rearrange("p (j c) -> p j c", j=J),
                in1=er_ps[:].rearrange("p (x c) -> p x c", x=1).to_broadcast([P, J, C]),
                op=MULT,
            )
            return bd

        def mix_quarter(b, X, bd, qs):
            """One output quarter: fp16 matmuls into [P,1024] PSUM tiles,
            ACT plain-copies to fp16 staging (normalization already folded
            into bd), a Pool-issued SWDGE DMA drains the quarter.  Two of
            the eight evacuation chunks per batch ride the otherwise-idle
            GpSimd so the ACT queue (the mix rate-setter) sheds ~2.2us."""
            out_b = out[b].rearrange("c (j n2) t -> j c (n2 t)", j=J)
            ost = opool.tile([P, OW], fp16, tag="ost")
            for h in range(OW // EW):
                mp = mixp.tile([P, EW], fp32, tag="mix")
                for g in range(EW // QW):
                    q0 = qs * OW + h * EW + g * QW
                    nc.tensor.matmul(
                        mp[:, g * QW : (g + 1) * QW],
                        lhsT=bd[:],
                        rhs=X[:, q0 : q0 + QW],
                        start=True,
                        stop=True,
                    )
                nc.scalar.copy(ost[:, h * EW : (h + 1) * EW], mp[:])
            nc.gpsimd.dma_start(
                out_b[:, :, qs * OW : (qs + 1) * OW],
                ost[:],
            )

        # Two-deep software pipeline, scheduled so each engine's in-order
        # queue almost never parks: iteration i mixes batch i (PE+ACT),
        # reduces batch i+1 (DVE tree then the score chain, whose PE/ACT
        # hops are interleaved between mix quarters at points where their
        # inputs are already computed), and streams batch i+2 in (SP DMA,
        # with the big alpha-multiply emitted last, after the DMA landed).
        Xs = {0: dma_in(0), 1: dma_in(1)}
        scrs = {0: mult(0, Xs[0])}
        s64s = {0: tree(0, scrs.pop(0))}
        bds = {0: chain_c(0, chain_b(0, chain_a(0, s64s.pop(0))))}
        scrs[1] = mult(1, Xs[1])
        for i in range(BS):
            if i + 2 < BS:
                Xs[i + 2] = dma_in(i + 2)
            nxt = i + 1 < BS
            if nxt:
                s64s[i + 1] = tree(i + 1, scrs.pop(i + 1))
            mix_quarter(i, Xs[i], bds[i], 0)
            mix_quarter(i, Xs[i], bds[i], 1)
            if nxt:
                sc_ps = chain_a(i + 1, s64s.pop(i + 1))
                att_sb = chain_b(i + 1, sc_ps)
            mix_quarter(i, Xs[i], bds[i], 2)
            if nxt:
                bds[i + 1] = chain_c(i + 1, att_sb)
            mix_quarter(i, Xs[i], bds.pop(i), 3)
            Xs.pop(i)
            if i + 2 < BS:
                scrs[i + 2] = mult(i + 2, Xs[i + 2])

    nc.compile()
    return nc


def _host_constants(Wc: np.ndarray, alpha: np.ndarray):
    # acx[(j*8+d), n2*64+t] = alpha[j*128+n2]  (pre-expanded so the DVE
    # multiply is a packed elementwise op eligible for the 2x fp16 mode)
    a = np.asarray(alpha, dtype=np.float32).reshape(J, N2)
    acx = np.repeat(a, C, axis=0).astype(np.float16)      # [(j,d), n2]
    # sel[(j*8+d), d'] = 1 if d == d'
    sel = np.tile(np.eye(C, dtype=np.float32), (J, 1))
    # rep[c', j*8+c] = 1 if c == c'  (partition replication)
    rep32 = np.tile(np.eye(C, dtype=np.float32), (1, J))
    a32 = np.zeros((P, 208), dtype=np.float32)
    a32[:, 0:8] = sel
    a32[:T, 8:72] = np.asarray(Wc, dtype=np.float32).T
    a32[:C, 72:80] = np.eye(C, dtype=np.float32)
    a32[:C, 80:208] = rep32
    # mask[(j,d), (j',c)] = 1 if j == j'
    mask = np.kron(np.eye(J, dtype=np.float16), np.ones((C, C), dtype=np.float16))
    a16 = np.zeros((P, 256), dtype=np.float16)
    a16[:C, 0:128] = rep32.astype(np.float16)
    a16[:, 128:256] = mask
    return {
        "acx": np.ascontiguousarray(acx),
        "a32": a32,
        "a16": a16,
    }


def get_program():
    if "nc" not in _PROGRAM_CACHE:
        _PROGRAM_CACHE["nc"] = _build_program()
    return _PROGRAM_CACHE["nc"]


def run(x, Wc, alpha, trace=False, trace_kwargs=None):
    """Run on 8 cores; returns (full_output, BassKernelResults)."""
    from concourse.bass_utils import run_bass_kernel_spmd

    nc = get_program()
    consts = _host_constants(np.asarray(Wc), np.asarray(alpha))
    xh = np.asarray(x).astype(np.float16)
    in_maps = []
    for r in range(NCORES):
        m = {"xs": np.ascontiguousarray(xh[r * BS : (r + 1) * BS])}
        m.update(consts)
        in_maps.append(m)
    kw = {}
    if trace:
        kw["trace"] = True
        if trace_kwargs:
            kw.update(trace_kwargs)
    res = run_bass_kernel_spmd(nc, in_maps, list(range(NCORES)), **kw)
    out = np.concatenate(
        [np.asarray(res.results[r]["out"]) for r in range(NCORES)], axis=0
    ).astype(np.float32)
    return out, res


def kernel(x, Wc, alpha):
    out, _ = run(x, Wc, alpha)
    return out.astype(np.float32)
